# revision 9
# baseline (speedup 1.0000x reference)
"""Trainium2 Bass kernel for MHSA with Transformer-XL relative position bias.

Problem: B=16, T=1024, DM=256, H=4, HS=64 fp32.
Sharding: pure data-parallel over batch across 8 cores (2 batches/core).

V2: phase 3 processes (b, head-pair) steps; the two heads of a pair live on
partition halves 0:64 / 64:128 of the same tiles, so their K=64 score
matmuls and M=64 AV matmuls run concurrently on the PE via auto
tile_position row/col tiling.  Pipeline stages per step s:
  stage_a(s):  pos scores X (paired MMs) -> PSUM -> f8 SBUF (DVE/ACT) ->
               sheared DRAM write (scalar HWDGE queue), rbf prefetch (gpsimd)
  stage_c(s-1): content (paired MMs) + shear add (identity f8 MM) ->
               exp on ACT [128,1024] with accum row-sums -> A bf16
  stage_n(s-1): reciprocal + normalize (DVE 4x) + xbar transpose -> at4
  stage_d(s-2): AV (col-paired MMs) -> avT evac
Out-proj per batch as soon as its AV steps drain.
"""
import sys

sys.path.insert(0, "/opt/trn_rl_repo")

import numpy as np

import concourse.bass as bass
import concourse.bacc as bacc
import concourse.tile as tile
from concourse import mybir
from concourse.masks import make_identity
from concourse.bass_utils import run_bass_kernel_spmd

B, T, DM, H, HS = 16, 1024, 256, 4, 64
NCORES = 8
BL = B // NCORES          # local batches per core (2)
M = BL * T                # local rows (2048)
NMT = M // 128            # m-tiles (16)
NT8 = T // 128            # m-tiles per batch (8)
P = 128
LN_EPS = 1e-3
F32 = mybir.dt.float32
BF16 = mybir.dt.bfloat16
F8 = mybir.dt.float8e4
EXPF = mybir.ActivationFunctionType.Exp


def build_bass():
    nc = bacc.Bacc("TRN2", target_bir_lowering=False, debug=False,
                   enable_asserts=False, num_devices=NCORES)

    x_in = nc.dram_tensor("x", [M, DM], F32, kind="ExternalInput").ap()
    pos_in = nc.dram_tensor("pos", [M, DM], F32, kind="ExternalInput").ap()
    wq_in = nc.dram_tensor("wq", [DM, DM], F32, kind="ExternalInput").ap()
    wk_in = nc.dram_tensor("wk", [DM, DM], F32, kind="ExternalInput").ap()
    wv_in = nc.dram_tensor("wv", [DM, DM], F32, kind="ExternalInput").ap()
    wp_in = nc.dram_tensor("wp", [DM, DM], F32, kind="ExternalInput").ap()
    wo_in = nc.dram_tensor("wo", [DM, DM], F32, kind="ExternalInput").ap()
    bqu_in = nc.dram_tensor("bqu", [DM], F32, kind="ExternalInput").ap()
    bqv_in = nc.dram_tensor("bqv", [DM], F32, kind="ExternalInput").ap()
    bk_in = nc.dram_tensor("bk", [DM], F32, kind="ExternalInput").ap()
    bo_in = nc.dram_tensor("bo", [DM], F32, kind="ExternalInput").ap()
    out = nc.dram_tensor("out", [M, DM], F32, kind="ExternalOutput").ap()

    # one scratch per (b, h): index 4*b + 2*hh + hp
    scr = [
        nc.dram_tensor(f"xscr{i}", [T, T + 1], F8, kind="Internal").ap()
        for i in range(BL * H)
    ]

    with tile.TileContext(nc) as tc:
        with tc.tile_pool(name="persist", bufs=1) as pp:
            # --- persistent SBUF ---
            ident = pp.tile([P, P], F32)
            make_identity(nc, ident)
            ident_f8 = pp.tile([P, P], F8)
            nc.vector.tensor_copy(out=ident_f8, in_=ident)

            def load_w(ap_in, name):
                # SWDGE cast-DMA: f32 DRAM -> bf16 SBUF directly
                ts = [pp.tile([P, DM], BF16, tag=f"{name}{c}", name=f"{name}{c}")
                      for c in range(2)]
                for c in range(2):
                    nc.gpsimd.dma_start(out=ts[c], in_=ap_in[c * P:(c + 1) * P, :])
                return ts

            posp_cm = tc.tile_pool(name="posp", bufs=1)
            posp = posp_cm.__enter__()
            pos_f32 = posp.tile([P, NMT, DM], F32, tag="pos_f32", name="pos_f32")
            posT = posp.tile([P, 2, M], BF16, tag="posT", name="posT")

            def load_pos(ch):
                nc.gpsimd.dma_start(
                    out=pos_f32[:, 4 * ch:4 * ch + 4, :],
                    in_=bass.AP(tensor=pos_in.tensor, offset=4 * ch * P * DM,
                                ap=[[DM, P], [P * DM, 4], [1, DM]]),
                )

            wq_sb = load_w(wq_in, "wq")
            wk_sb = load_w(wk_in, "wk")
            wp_sb = load_w(wp_in, "wp")
            load_pos(0)
            load_pos(1)
            wv_sb = load_w(wv_in, "wv")
            load_pos(2)
            load_pos(3)
            wo_sb = load_w(wo_in, "wo")

            def load_col(ap_in, name):
                ts = [pp.tile([P, 1], F32, tag=f"{name}{c}", name=f"{name}{c}") for c in range(2)]
                for c in range(2):
                    nc.gpsimd.dma_start(
                        out=ts[c],
                        in_=bass.AP(tensor=ap_in.tensor, offset=c * P, ap=[[1, P], [1, 1]]),
                    )
                return ts

            bqu_c = load_col(bqu_in, "bqu")
            bqv_c = load_col(bqv_in, "bqv")
            bk_c = load_col(bk_in, "bk")
            dqv_c = [pp.tile([P, 1], F32, tag=f"dqv{c}", name=f"dqv{c}") for c in range(2)]
            for c in range(2):
                nc.vector.tensor_tensor(out=dqv_c[c], in0=bqv_c[c], in1=bqu_c[c],
                                        op=mybir.AluOpType.subtract)

            bo_b = pp.tile([P, DM], F32, tag="bo_b", name="bo_b")
            nc.gpsimd.dma_start(
                out=bo_b,
                in_=bass.AP(tensor=bo_in.tensor, offset=0, ap=[[0, P], [1, DM]]),
            )

            eps_t = pp.tile([P, 1], F32)
            nc.vector.memset(eps_t, LN_EPS)

            # zero column 0 of all scratch buffers (writes never touch it)
            zcol = pp.tile([P, 8], F8, tag="zcol", name="zcol")
            nc.vector.memset(zcol, 0.0)
            for i in range(BL * H):
                nc.gpsimd.dma_start(
                    out=bass.AP(tensor=scr[i].tensor, offset=0,
                                ap=[[T + 1, P], [P * (T + 1), 8]]),
                    in_=zcol,
                )

            x_res = pp.tile([P, NMT, DM], F32)        # residual copy of inputs
            xnT = pp.tile([P, 2, M], BF16, tag="xnT", name="xnT")
            # per-batch projection tiles: [sc][b] -> [P, T]
            def per_b(name, cols=T):
                return [[pp.tile([P, cols], BF16, tag=f"{name}{c}b{b}",
                                 name=f"{name}{c}b{b}") for b in range(BL)]
                        for c in range(2)]

            quT = per_b("quT")
            qvT = per_b("qvT")
            kT = per_b("kT")
            pT = per_b("pT")
            avT = per_b("avT")
            v_sb = [pp.tile([P, NT8, DM], BF16, tag=f"v_b{b}", name=f"v_b{b}")
                    for b in range(BL)]

            # x loads in 4 chunks so LN can start after the first chunk
            for ch in range(4):
                nc.sync.dma_start(
                    out=x_res[:, 4 * ch:4 * ch + 4, :],
                    in_=bass.AP(tensor=x_in.tensor, offset=4 * ch * P * DM,
                                ap=[[DM, P], [P * DM, 4], [1, DM]]),
                )

            # ------- phases 1+2 merged per 4-mt chunk (PE stays dense) -----
            if True:
                with tc.tile_pool(name="ph1", bufs=8) as sb1, \
                     tc.tile_pool(name="ps1", bufs=2, space="PSUM") as ps1, \
                     tc.tile_pool(name="ps2", bufs=2, space="PSUM") as ps2:
                    for ch in range(4):
                        b = ch // 2
                        mts = list(range(4 * ch, 4 * ch + 4))
                        mvs = {}; rstds = {}
                        for mt in mts:
                            stats = sb1.tile([P, 6], F32, tag="stats")
                            nc.vector.bn_stats(out=stats, in_=x_res[:, mt, :])
                            mv = sb1.tile([P, 2], F32, tag="mv")
                            nc.vector.bn_aggr(out=mv, in_=stats)
                            mvs[mt] = mv
                        for mt in mts:
                            rstd = sb1.tile([P, 1], F32, tag="rstd")
                            nc.scalar.activation(out=rstd, in_=mvs[mt][:, 1:2],
                                                 func=mybir.ActivationFunctionType.Sqrt,
                                                 bias=eps_t, scale=1.0)
                            rstds[mt] = rstd
                        for mt in mts:
                            nc.vector.reciprocal(out=rstds[mt], in_=rstds[mt])
                        for mt in mts:
                            xn = sb1.tile([P, DM], F32, tag="xn")
                            nc.vector.tensor_scalar(out=xn, in0=x_res[:, mt, :],
                                                    scalar1=mvs[mt][:, 0:1],
                                                    scalar2=rstds[mt],
                                                    op0=mybir.AluOpType.subtract,
                                                    op1=mybir.AluOpType.mult)
                            for c in range(2):
                                tp = ps1.tile([P, P], F32, tag="tp")
                                nc.tensor.transpose(tp, xn[:, c * P:(c + 1) * P], ident)
                                if c == 0:
                                    nc.scalar.copy(out=xnT[:, c, mt * P:(mt + 1) * P],
                                                   in_=tp)
                                else:
                                    nc.vector.tensor_copy(
                                        out=xnT[:, c, mt * P:(mt + 1) * P], in_=tp)
                        for mt in mts:
                            ptb = sb1.tile([P, DM], BF16, tag="ptb")
                            nc.vector.tensor_copy(out=ptb, in_=pos_f32[:, mt, :])
                            nc.sync.dma_start_transpose(
                                out=posT[:, :, mt * P:(mt + 1) * P], in_=ptb)
                        # projections for this chunk's m-range
                        msl = slice(ch * 512, (ch + 1) * 512)      # global m
                        bsl = slice((ch % 2) * 512, (ch % 2) * 512 + 512)  # within b
                        for sc in range(2):
                            pq = ps2.tile([P, 512], F32, tag="pq")
                            pk = ps2.tile([P, 512], F32, tag="pk")
                            pps = ps2.tile([P, 512], F32, tag="pp")
                            for dc in range(2):
                                nc.tensor.matmul(pq, lhsT=wq_sb[dc][:, sc * P:(sc + 1) * P],
                                                 rhs=xnT[:, dc, msl],
                                                 start=(dc == 0), stop=(dc == 1))
                                nc.tensor.matmul(pk, lhsT=wk_sb[dc][:, sc * P:(sc + 1) * P],
                                                 rhs=xnT[:, dc, msl],
                                                 start=(dc == 0), stop=(dc == 1))
                                nc.tensor.matmul(pps, lhsT=wp_sb[dc][:, sc * P:(sc + 1) * P],
                                                 rhs=posT[:, dc, msl],
                                                 start=(dc == 0), stop=(dc == 1))
                            nc.scalar.activation(out=quT[sc][b][:, bsl], in_=pq,
                                                 func=mybir.ActivationFunctionType.Identity,
                                                 bias=bqu_c[sc], scale=1.0)
                            nc.vector.tensor_scalar_add(out=qvT[sc][b][:, bsl],
                                                        in0=quT[sc][b][:, bsl],
                                                        scalar1=dqv_c[sc])
                            nc.scalar.activation(out=kT[sc][b][:, bsl], in_=pk,
                                                 func=mybir.ActivationFunctionType.Identity,
                                                 bias=bk_c[sc], scale=1.0)
                            nc.vector.tensor_copy(out=pT[sc][b][:, bsl], in_=pps)
                        for mt in mts:
                            pv = ps2.tile([P, 512], F32, tag="pq")
                            for dc in range(2):
                                nc.tensor.matmul(pv[:, :DM],
                                                 lhsT=xnT[:, dc, mt * P:(mt + 1) * P],
                                                 rhs=wv_sb[dc],
                                                 start=(dc == 0), stop=(dc == 1))
                            nc.vector.tensor_copy(out=v_sb[b][:, mt % NT8, :],
                                                  in_=pv[:, :DM])

            posp_cm.__exit__(None, None, None)  # free pos staging SBUF

            # ---------------- phase 3: attention per (b, head-pair) --------
            NST = BL * 2  # 4 steps, each covers heads (2*hh, 2*hh+1)
            with tc.tile_pool(name="ph3", bufs=8) as sb3, \
                 tc.tile_pool(name="ssp", bufs=20) as ssp, \
                 tc.tile_pool(name="abp", bufs=4) as abp, \
                 tc.tile_pool(name="at", bufs=4) as atp, \
                 tc.tile_pool(name="xfp", bufs=4) as xfp, \
                 tc.tile_pool(name="psX", bufs=2, space="PSUM") as psX, \
                 tc.tile_pool(name="psC", bufs=2, space="PSUM") as psC, \
                 tc.tile_pool(name="psAV", bufs=2, space="PSUM") as psAV:

                xbf_t = {}     # (s, hp, q) -> X staging f8 [P, 2, T]
                rbf_t = {}     # (s, hp, q) -> sheared re-read f8 [P, 2, T]
                ab_t = {}      # (s, hp, q) -> A bf16 [P, 2, T]
                at_t = {}      # (s, hp) -> at4 bf16 [P, NT8, NT8, P]
                cp_t = {}      # (s, hp, mt) -> content PSUM [P, T]
                ssum_t = {}    # (s, hp, mt) -> row sums [P, 1]

                def sdec(s):
                    b, hh = divmod(s, 2)
                    return b, hh

                def scr_i(s, hp):
                    b, hh = sdec(s)
                    return 4 * b + 2 * hh + hp

                def stage_a_mm(s, mt):
                    # pos scores for both heads of the pair; evac + stage
                    b, hh = sdec(s)
                    mg = slice(mt * P, (mt + 1) * P)
                    q = mt // 2
                    for hp in range(2):
                        if mt % 2 == 0:
                            xbf_t[(s, hp, q)] = xfp.tile([P, 2, T], F8,
                                                         tag="xbf", name="xbf")
                    for nck in range(2):
                        nsl = slice(nck * 512, (nck + 1) * 512)
                        xps = []
                        for hp in range(2):
                            po = hp * 64
                            ssl = slice(po, po + 64)
                            xp = psX.tile([P, 512], F32, tag="xp", name="xp")
                            nc.tensor.matmul(xp, lhsT=qvT[hh][b][ssl, mg],
                                             rhs=pT[hh][b][ssl, nsl],
                                             start=True, stop=True)
                            xps.append(xp)
                        for hp in range(2):
                            osl = xbf_t[(s, hp, q)][:, mt % 2, nsl]
                            # ACT takes a small share of the evacs
                            if hp == 0 and nck == 0 and mt % 2 == 1:
                                nc.scalar.copy(out=osl, in_=xps[hp])
                            else:
                                nc.vector.tensor_copy(out=osl, in_=xps[hp])

                def stage_a_wr(s, q):
                    # sheared write of mt pair (2q, 2q+1) + rbf prefetches
                    for hp in range(2):
                        sc_t = scr[scr_i(s, hp)]
                        nc.scalar.dma_start(
                            out=bass.AP(tensor=sc_t.tensor,
                                        offset=2 * q * P * (T + 1) + 1,
                                        ap=[[T + 1, P], [P * (T + 1), 2], [1, T]]),
                            in_=xbf_t.pop((s, hp, q)))
                    for hp in range(2):
                        if q >= 1:
                            prefetch_rbf(s, hp, q - 1)
                        if q == 3:
                            prefetch_rbf(s, hp, 3)

                def prefetch_rbf(s, hp, q):
                    sc_t = scr[scr_i(s, hp)]
                    rbf2 = sb3.tile([P, 2, T], F8, tag="rbf", name="rbf")
                    nc.gpsimd.dma_start(
                        out=rbf2,
                        in_=bass.AP(tensor=sc_t.tensor, offset=T + 2 * q * P * T,
                                    ap=[[T, P], [P * T, 2], [1, T]]))
                    rbf_t[(s, hp, q)] = rbf2

                def stage_c(s, mt):
                    # content (paired) + shear add + exp for both heads
                    b, hh = sdec(s)
                    mg = slice(mt * P, (mt + 1) * P)
                    q = mt // 2
                    cps = []
                    for hp in range(2):
                        cp = psC.tile([P, T], F32, tag="cp", name="cp")
                        cp_t[(s, hp, mt)] = cp
                        cps.append(cp)
                    for nck in range(2):
                        nsl = slice(nck * 512, (nck + 1) * 512)
                        for hp in range(2):
                            po = hp * 64
                            ssl = slice(po, po + 64)
                            nc.tensor.matmul(cps[hp][:, nsl],
                                             lhsT=quT[hh][b][ssl, mg],
                                             rhs=kT[hh][b][ssl, nsl],
                                             start=True, stop=False)
                    for hp in range(2):
                        rbf2 = rbf_t[(s, hp, q)]
                        for nck in range(2):
                            nsl = slice(nck * 512, (nck + 1) * 512)
                            nc.tensor.matmul(cps[hp][:, nsl], lhsT=ident_f8,
                                             rhs=rbf2[:, mt % 2, nsl],
                                             start=False, stop=(nck == 1))
                        if mt % 2 == 1:
                            del rbf_t[(s, hp, q)]
                    for hp in range(2):
                        if mt % 2 == 0:
                            ab_t[(s, hp, q)] = abp.tile([P, 2, T], BF16,
                                                        tag="ab", name="ab")
                        half = ab_t[(s, hp, q)][:, mt % 2, :]
                        ssum = ssp.tile([P, 1], F32, tag="ssum", name="ssum")
                        nc.scalar.activation(out=half, in_=cps[hp], func=EXPF,
                                             scale=0.125, accum_out=ssum)
                        ssum_t[(s, hp, mt)] = ssum

                def stage_n(s, mt):
                    # normalize A rows by 1/rowsum; xbar at pair boundary
                    q = mt // 2
                    for hp in range(2):
                        ssum = ssum_t.pop((s, hp, mt))
                        nc.vector.reciprocal(out=ssum, in_=ssum)
                        half = ab_t[(s, hp, q)][:, mt % 2, :]
                        nc.vector.tensor_scalar_mul(out=half, in0=half, scalar1=ssum)
                        del cp_t[(s, hp, mt)]
                    if mt % 2 == 1:
                        for hp in range(2):
                            at4 = at_t[(s, hp)]
                            nc.sync.dma_start_transpose(
                                out=at4[:, 2 * q:2 * q + 2, :, :],
                                in_=ab_t.pop((s, hp, q)))

                def stage_d(s):
                    # AV for both heads, col-paired: hp0 -> psum rows 0:64,
                    # hp1 -> rows 64:128 (concurrent col strips)
                    b, hh = sdec(s)
                    avps = [psAV.tile([P, 512], F32, tag="av", name=f"avp{mc}")
                            for mc in range(2)]
                    for nt in range(NT8):
                        for mc in range(2):
                            for hp in range(2):
                                h = 2 * hh + hp
                                po = hp * 64
                                nc.tensor.matmul(
                                    avps[mc][po:po + 64, :],
                                    lhsT=v_sb[b][:, nt, h * HS:(h + 1) * HS],
                                    rhs=at_t[(s, hp)][:, 4 * mc:4 * mc + 4, nt, :],
                                    start=(nt == 0), stop=(nt == NT8 - 1))
                    for hp in range(2):
                        del at_t[(s, hp)]
                    for mc in range(2):
                        nc.vector.tensor_copy(
                            out=avT[hh][b][:, mc * 512:(mc + 1) * 512],
                            in_=avps[mc])

                def out_proj(b):
                    with tc.tile_pool(name=f"ph4_{b}", bufs=3) as sb4, \
                         tc.tile_pool(name=f"ps4_{b}", bufs=2, space="PSUM") as ps4:
                        for mt8 in range(NT8):
                            mt = b * NT8 + mt8
                            op = ps4.tile([P, DM], F32, tag="op")
                            for sc in range(2):
                                nc.tensor.matmul(
                                    op,
                                    lhsT=avT[sc][b][:, mt8 * P:(mt8 + 1) * P],
                                    rhs=wo_sb[sc],
                                    start=(sc == 0), stop=(sc == 1))
                            ot = sb4.tile([P, DM], F32, tag="ot")
                            nc.vector.scalar_tensor_tensor(
                                out=ot, in0=op, scalar=0.0, in1=x_res[:, mt, :],
                                op0=mybir.AluOpType.bypass, op1=mybir.AluOpType.add)
                            nc.vector.tensor_tensor(out=ot, in0=ot, in1=bo_b,
                                                    op=mybir.AluOpType.add)
                            nc.scalar.dma_start(out=out[mt * P:(mt + 1) * P, :],
                                                in_=ot)

                for step in range(NST + 2):
                    if 0 <= step - 1 < NST:
                        for hp in range(2):
                            at_t[(step - 1, hp)] = atp.tile(
                                [P, NT8, NT8, P], BF16, tag="at", name="at")
                    for mt in range(NT8):
                        if 0 <= step - 1 < NST:
                            stage_c(step - 1, mt)
                            if mt >= 2:
                                stage_n(step - 1, mt - 2)
                        if step < NST:
                            stage_a_mm(step, mt)
                            if mt % 2 == 1:
                                stage_a_wr(step, mt // 2)
                    if 0 <= step - 1 < NST:
                        stage_n(step - 1, NT8 - 2)
                        stage_n(step - 1, NT8 - 1)
                    if step - 2 >= 0:
                        stage_d(step - 2)

            # ---------------- phase 4: out-proj + residual ----------------
            out_proj(0)
            out_proj(1)
    nc.finalize()
    return nc


_NC = None


def make_in_maps(inputs):
    f = lambda a: np.ascontiguousarray(np.asarray(a, dtype=np.float32))
    x = f(inputs["inputs"]).reshape(B, T, DM)
    pos = f(inputs["pos_enc"]).reshape(B, T, DM)
    wq0 = f(inputs["Wq"]).reshape(DM, DM)
    wk0 = f(inputs["Wk"]).reshape(DM, DM)
    wv0 = f(inputs["Wv"]).reshape(DM, DM)
    wp = f(inputs["Wp"]).reshape(DM, DM)
    wo = f(inputs["Wo"]).reshape(DM, DM)
    gamma = f(inputs["gamma"]).reshape(DM, 1)
    beta = f(inputs["beta"]).reshape(DM)
    # fold LN's gamma into the x-side weights, beta into the projection biases,
    # and bv through softmax (rows sum to 1) into the output bias
    wq, wk, wv = gamma * wq0, gamma * wk0, gamma * wv0
    bqu = (f(inputs["bq"]).reshape(DM) + f(inputs["pos_bias_u"]).reshape(DM)
           + beta @ wq0)
    bqv = (f(inputs["bq"]).reshape(DM) + f(inputs["pos_bias_v"]).reshape(DM)
           + beta @ wq0)
    bk = f(inputs["bk"]).reshape(DM) + beta @ wk0
    bv_eff = f(inputs["bv"]).reshape(DM) + beta @ wv0
    bo = f(inputs["bo"]) + bv_eff @ wo
    shared = dict(
        wq=wq, wk=wk, wv=wv, wp=wp, wo=wo,
        bqu=bqu, bqv=bqv, bk=bk, bo=bo,
    )
    in_maps = []
    for c in range(NCORES):
        sl = slice(c * BL, (c + 1) * BL)
        in_maps.append(dict(
            x=np.ascontiguousarray(x[sl].reshape(M, DM)),
            pos=np.ascontiguousarray(pos[sl].reshape(M, DM)),
            **shared,
        ))
    return in_maps


def kernel(**inputs) -> np.ndarray:
    global _NC
    if _NC is None:
        _NC = build_bass()
    in_maps = make_in_maps(inputs)
    res = run_bass_kernel_spmd(_NC, in_maps, core_ids=list(range(NCORES)))
    outs = [r["out"].reshape(BL, T, DM) for r in res.results]
    return np.concatenate(outs, axis=0)


# revision 13
# speedup vs baseline: 1.1977x; 1.1977x over previous
"""Trainium2 Bass kernel for MHSA with Transformer-XL relative position bias.

Problem: B=16, T=1024, DM=256, H=4, HS=64 fp32.
Sharding: pure data-parallel over batch across 8 cores (2 batches/core).

V2: phase 3 processes (b, head-pair) steps; the two heads of a pair live on
partition halves 0:64 / 64:128 of the same tiles, so their K=64 score
matmuls and M=64 AV matmuls run concurrently on the PE via auto
tile_position row/col tiling.  Pipeline stages per step s:
  stage_a(s):  pos scores X (paired MMs) -> PSUM -> f8 SBUF (DVE/ACT) ->
               sheared DRAM write (scalar HWDGE queue), rbf prefetch (gpsimd)
  stage_c(s-1): content (paired MMs) + shear add (identity f8 MM) ->
               exp on ACT [128,1024] with accum row-sums -> A bf16
  stage_n(s-1): reciprocal + normalize (DVE 4x) + xbar transpose -> at4
  stage_d(s-2): AV (col-paired MMs) -> avT evac
Out-proj per batch as soon as its AV steps drain.
"""
import sys

sys.path.insert(0, "/opt/trn_rl_repo")

import numpy as np

import concourse.bass as bass
import concourse.bacc as bacc
import concourse.tile as tile
from concourse import mybir
from concourse.masks import make_identity
from concourse.bass_utils import run_bass_kernel_spmd

B, T, DM, H, HS = 16, 1024, 256, 4, 64
NCORES = 8
BL = B // NCORES          # local batches per core (2)
M = BL * T                # local rows (2048)
NMT = M // 128            # m-tiles (16)
NT8 = T // 128            # m-tiles per batch (8)
P = 128
LN_EPS = 1e-3
F32 = mybir.dt.float32
BF16 = mybir.dt.bfloat16
F8 = mybir.dt.float8e4
EXPF = mybir.ActivationFunctionType.Exp


def build_bass():
    nc = bacc.Bacc("TRN2", target_bir_lowering=False, debug=False,
                   enable_asserts=False, num_devices=NCORES)

    x_in = nc.dram_tensor("x", [M, DM], F32, kind="ExternalInput").ap()
    pos_in = nc.dram_tensor("pos", [M, DM], F32, kind="ExternalInput").ap()
    wq_in = nc.dram_tensor("wq", [DM, DM], F32, kind="ExternalInput").ap()
    wk_in = nc.dram_tensor("wk", [DM, DM], F32, kind="ExternalInput").ap()
    wv_in = nc.dram_tensor("wv", [DM, DM], F32, kind="ExternalInput").ap()
    wp_in = nc.dram_tensor("wp", [DM, DM], F32, kind="ExternalInput").ap()
    wo_in = nc.dram_tensor("wo", [DM, DM], F32, kind="ExternalInput").ap()
    bqu_in = nc.dram_tensor("bqu", [DM], F32, kind="ExternalInput").ap()
    bqv_in = nc.dram_tensor("bqv", [DM], F32, kind="ExternalInput").ap()
    bk_in = nc.dram_tensor("bk", [DM], F32, kind="ExternalInput").ap()
    bo_in = nc.dram_tensor("bo", [DM], F32, kind="ExternalInput").ap()
    out = nc.dram_tensor("out", [M, DM], F32, kind="ExternalOutput").ap()

    # one scratch per (b, h): index 4*b + 2*hh + hp
    scr = [
        nc.dram_tensor(f"xscr{i}", [T, T + 1], F8, kind="Internal").ap()
        for i in range(BL * H)
    ]

    with tile.TileContext(nc) as tc:
        with tc.tile_pool(name="persist", bufs=1) as pp:
            # --- persistent SBUF ---
            ident = pp.tile([P, P], F32)
            make_identity(nc, ident)
            ident_f8 = pp.tile([P, P], F8)
            nc.vector.tensor_copy(out=ident_f8, in_=ident)

            def load_w(ap_in, name):
                # SWDGE cast-DMA: f32 DRAM -> bf16 SBUF directly
                ts = [pp.tile([P, DM], BF16, tag=f"{name}{c}", name=f"{name}{c}")
                      for c in range(2)]
                for c in range(2):
                    nc.gpsimd.dma_start(out=ts[c], in_=ap_in[c * P:(c + 1) * P, :])
                return ts

            posp_cm = tc.tile_pool(name="posp", bufs=1)
            posp = posp_cm.__enter__()
            pos_f32 = posp.tile([P, NMT, DM], F32, tag="pos_f32", name="pos_f32")
            posT = posp.tile([P, 2, M], BF16, tag="posT", name="posT")

            def load_pos(ch):
                nc.gpsimd.dma_start(
                    out=pos_f32[:, 4 * ch:4 * ch + 4, :],
                    in_=bass.AP(tensor=pos_in.tensor, offset=4 * ch * P * DM,
                                ap=[[DM, P], [P * DM, 4], [1, DM]]),
                )

            wq_sb = load_w(wq_in, "wq")
            wk_sb = load_w(wk_in, "wk")
            wp_sb = load_w(wp_in, "wp")
            load_pos(0)
            load_pos(1)
            wv_sb = load_w(wv_in, "wv")
            load_pos(2)
            load_pos(3)
            wo_sb = load_w(wo_in, "wo")

            def load_col(ap_in, name):
                ts = [pp.tile([P, 1], F32, tag=f"{name}{c}", name=f"{name}{c}") for c in range(2)]
                for c in range(2):
                    nc.gpsimd.dma_start(
                        out=ts[c],
                        in_=bass.AP(tensor=ap_in.tensor, offset=c * P, ap=[[1, P], [1, 1]]),
                    )
                return ts

            bqu_c = load_col(bqu_in, "bqu")
            bqv_c = load_col(bqv_in, "bqv")
            bk_c = load_col(bk_in, "bk")
            dqv_c = [pp.tile([P, 1], F32, tag=f"dqv{c}", name=f"dqv{c}") for c in range(2)]
            for c in range(2):
                nc.vector.tensor_tensor(out=dqv_c[c], in0=bqv_c[c], in1=bqu_c[c],
                                        op=mybir.AluOpType.subtract)

            bo_b = pp.tile([P, DM], F32, tag="bo_b", name="bo_b")
            nc.gpsimd.dma_start(
                out=bo_b,
                in_=bass.AP(tensor=bo_in.tensor, offset=0, ap=[[0, P], [1, DM]]),
            )

            eps_t = pp.tile([P, 1], F32)
            nc.vector.memset(eps_t, LN_EPS)

            # zero column 0 of all scratch buffers (writes never touch it)
            zcol = pp.tile([P, 8], F8, tag="zcol", name="zcol")
            nc.vector.memset(zcol, 0.0)
            for i in range(BL * H):
                nc.gpsimd.dma_start(
                    out=bass.AP(tensor=scr[i].tensor, offset=0,
                                ap=[[T + 1, P], [P * (T + 1), 8]]),
                    in_=zcol,
                )

            x_res = pp.tile([P, NMT, DM], F32)        # residual copy of inputs
            xnT = pp.tile([P, 2, M], BF16, tag="xnT", name="xnT")
            # per-batch projection tiles: [sc][b] -> [P, T]
            def per_b(name, cols=T):
                return [[pp.tile([P, cols], BF16, tag=f"{name}{c}b{b}",
                                 name=f"{name}{c}b{b}") for b in range(BL)]
                        for c in range(2)]

            quT = per_b("quT")
            qvT = per_b("qvT")
            kT = per_b("kT")
            pT = per_b("pT")
            avT = per_b("avT")
            v_sb = [pp.tile([P, NT8, DM], BF16, tag=f"v_b{b}", name=f"v_b{b}")
                    for b in range(BL)]

            # x loads in 4 chunks so LN can start after the first chunk
            for ch in range(4):
                nc.sync.dma_start(
                    out=x_res[:, 4 * ch:4 * ch + 4, :],
                    in_=bass.AP(tensor=x_in.tensor, offset=4 * ch * P * DM,
                                ap=[[DM, P], [P * DM, 4], [1, DM]]),
                )

            # ------- phases 1+2 merged per 4-mt chunk (PE stays dense) -----
            if True:
                with tc.tile_pool(name="ph1", bufs=8) as sb1, \
                     tc.tile_pool(name="ps1", bufs=2, space="PSUM") as ps1, \
                     tc.tile_pool(name="ps2", bufs=2, space="PSUM") as ps2:
                    for ch in range(4):
                        b = ch // 2
                        mts = list(range(4 * ch, 4 * ch + 4))
                        mvs = {}; rstds = {}
                        for mt in mts:
                            stats = sb1.tile([P, 6], F32, tag="stats")
                            nc.vector.bn_stats(out=stats, in_=x_res[:, mt, :])
                            mv = sb1.tile([P, 2], F32, tag="mv")
                            nc.vector.bn_aggr(out=mv, in_=stats)
                            mvs[mt] = mv
                        for mt in mts:
                            rstd = sb1.tile([P, 1], F32, tag="rstd")
                            nc.scalar.activation(out=rstd, in_=mvs[mt][:, 1:2],
                                                 func=mybir.ActivationFunctionType.Sqrt,
                                                 bias=eps_t, scale=1.0)
                            rstds[mt] = rstd
                        for mt in mts:
                            nc.vector.reciprocal(out=rstds[mt], in_=rstds[mt])
                        for mt in mts:
                            xn = sb1.tile([P, DM], F32, tag="xn")
                            nc.vector.tensor_scalar(out=xn, in0=x_res[:, mt, :],
                                                    scalar1=mvs[mt][:, 0:1],
                                                    scalar2=rstds[mt],
                                                    op0=mybir.AluOpType.subtract,
                                                    op1=mybir.AluOpType.mult)
                            for c in range(2):
                                tp = ps1.tile([P, P], F32, tag="tp")
                                nc.tensor.transpose(tp, xn[:, c * P:(c + 1) * P], ident)
                                if c == 0:
                                    nc.scalar.copy(out=xnT[:, c, mt * P:(mt + 1) * P],
                                                   in_=tp)
                                else:
                                    nc.vector.tensor_copy(
                                        out=xnT[:, c, mt * P:(mt + 1) * P], in_=tp)
                        for mt in mts:
                            ptb = sb1.tile([P, DM], BF16, tag="ptb")
                            nc.vector.tensor_copy(out=ptb, in_=pos_f32[:, mt, :])
                            nc.sync.dma_start_transpose(
                                out=posT[:, :, mt * P:(mt + 1) * P], in_=ptb)
                        # projections for this chunk's m-range
                        msl = slice(ch * 512, (ch + 1) * 512)      # global m
                        bsl = slice((ch % 2) * 512, (ch % 2) * 512 + 512)  # within b
                        for sc in range(2):
                            pq = ps2.tile([P, 512], F32, tag="pq")
                            pk = ps2.tile([P, 512], F32, tag="pk")
                            pps = ps2.tile([P, 512], F32, tag="pp")
                            for dc in range(2):
                                nc.tensor.matmul(pq, lhsT=wq_sb[dc][:, sc * P:(sc + 1) * P],
                                                 rhs=xnT[:, dc, msl],
                                                 start=(dc == 0), stop=(dc == 1))
                                nc.tensor.matmul(pk, lhsT=wk_sb[dc][:, sc * P:(sc + 1) * P],
                                                 rhs=xnT[:, dc, msl],
                                                 start=(dc == 0), stop=(dc == 1))
                                nc.tensor.matmul(pps, lhsT=wp_sb[dc][:, sc * P:(sc + 1) * P],
                                                 rhs=posT[:, dc, msl],
                                                 start=(dc == 0), stop=(dc == 1))
                            nc.scalar.activation(out=quT[sc][b][:, bsl], in_=pq,
                                                 func=mybir.ActivationFunctionType.Identity,
                                                 bias=bqu_c[sc], scale=1.0)
                            nc.vector.tensor_scalar_add(out=qvT[sc][b][:, bsl],
                                                        in0=quT[sc][b][:, bsl],
                                                        scalar1=dqv_c[sc])
                            nc.scalar.activation(out=kT[sc][b][:, bsl], in_=pk,
                                                 func=mybir.ActivationFunctionType.Identity,
                                                 bias=bk_c[sc], scale=1.0)
                            nc.vector.tensor_copy(out=pT[sc][b][:, bsl], in_=pps)
                        for mt in mts:
                            pv = ps2.tile([P, 512], F32, tag="pq")
                            for dc in range(2):
                                nc.tensor.matmul(pv[:, :DM],
                                                 lhsT=xnT[:, dc, mt * P:(mt + 1) * P],
                                                 rhs=wv_sb[dc],
                                                 start=(dc == 0), stop=(dc == 1))
                            nc.vector.tensor_copy(out=v_sb[b][:, mt % NT8, :],
                                                  in_=pv[:, :DM])

            posp_cm.__exit__(None, None, None)  # free pos staging SBUF

            # ---------------- phase 3: attention per (b, head-pair) --------
            NST = BL * 2  # 4 steps, each covers heads (2*hh, 2*hh+1)
            with tc.tile_pool(name="ph3", bufs=8) as sb3, \
                 tc.tile_pool(name="ssp", bufs=20) as ssp, \
                 tc.tile_pool(name="abp", bufs=4) as abp, \
                 tc.tile_pool(name="at", bufs=4) as atp, \
                 tc.tile_pool(name="xfp", bufs=4) as xfp, \
                 tc.tile_pool(name="psX", bufs=2, space="PSUM") as psX, \
                 tc.tile_pool(name="psC", bufs=3, space="PSUM") as psC:

                xbf_t = {}     # (s, hp, q) -> X staging f8 [P, 2, T]
                rbf_t = {}     # (s, hp, q) -> sheared re-read f8 [P, 2, T]
                ab_t = {}      # (s, hp, q) -> A bf16 [P, 2, T]
                at_t = {}      # (s, hp) -> at4 bf16 [P, NT8, NT8, P]
                cp_t = {}      # (s, hp, mt) -> content PSUM [P, T]
                ssum_t = {}    # (s, hp, mt) -> row sums [P, 1]

                def sdec(s):
                    b, hh = divmod(s, 2)
                    return b, hh

                def scr_i(s, hp):
                    b, hh = sdec(s)
                    return 4 * b + 2 * hh + hp

                def stage_a_mm(s, mt):
                    # pos scores for both heads of the pair; evac + stage.
                    # hp-outer so the two matmuls per hp share one LDWEIGHTS.
                    b, hh = sdec(s)
                    mg = slice(mt * P, (mt + 1) * P)
                    q = mt // 2
                    for hp in range(2):
                        if mt % 2 == 0:
                            xbf_t[(s, hp, q)] = xfp.tile([P, 2, T], F8,
                                                         tag="xbf", name="xbf")
                    for hp in range(2):
                        po = hp * 64
                        ssl = slice(po, po + 64)
                        xps = []
                        for nck in range(2):
                            nsl = slice(nck * 512, (nck + 1) * 512)
                            xp = psX.tile([P, 512], F32, tag="xp", name="xp")
                            nc.tensor.matmul(xp, lhsT=qvT[hh][b][ssl, mg],
                                             rhs=pT[hh][b][ssl, nsl],
                                             start=True, stop=True)
                            xps.append(xp)
                        for nck in range(2):
                            osl = xbf_t[(s, hp, q)][:, mt % 2,
                                                    nck * 512:(nck + 1) * 512]
                            # ACT takes a small share of the evacs
                            if hp == 0 and nck == 0 and mt % 2 == 1:
                                nc.scalar.copy(out=osl, in_=xps[nck])
                            else:
                                nc.vector.tensor_copy(out=osl, in_=xps[nck])

                def stage_a_wr(s, q):
                    # sheared write of mt pair (2q, 2q+1) + rbf prefetches
                    for hp in range(2):
                        sc_t = scr[scr_i(s, hp)]
                        nc.scalar.dma_start(
                            out=bass.AP(tensor=sc_t.tensor,
                                        offset=2 * q * P * (T + 1) + 1,
                                        ap=[[T + 1, P], [P * (T + 1), 2], [1, T]]),
                            in_=xbf_t.pop((s, hp, q)))
                    for hp in range(2):
                        if q >= 1:
                            prefetch_rbf(s, hp, q - 1)
                        if q == 3:
                            prefetch_rbf(s, hp, 3)

                def prefetch_rbf(s, hp, q):
                    sc_t = scr[scr_i(s, hp)]
                    rbf2 = sb3.tile([P, 2, T], F8, tag="rbf", name="rbf")
                    nc.gpsimd.dma_start(
                        out=rbf2,
                        in_=bass.AP(tensor=sc_t.tensor, offset=T + 2 * q * P * T,
                                    ap=[[T, P], [P * T, 2], [1, T]]))
                    rbf_t[(s, hp, q)] = rbf2

                def stage_c(s, hp, mt):
                    # content + shear add + exp for one head of the pair
                    b, hh = sdec(s)
                    mg = slice(mt * P, (mt + 1) * P)
                    q = mt // 2
                    po = hp * 64
                    ssl = slice(po, po + 64)
                    cp = psC.tile([P, T], F32, tag="cp", name="cp")
                    for nck in range(2):
                        nsl = slice(nck * 512, (nck + 1) * 512)
                        nc.tensor.matmul(cp[:, nsl],
                                         lhsT=quT[hh][b][ssl, mg],
                                         rhs=kT[hh][b][ssl, nsl],
                                         start=True, stop=False)
                    rbf2 = rbf_t[(s, hp, q)]
                    for nck in range(2):
                        nsl = slice(nck * 512, (nck + 1) * 512)
                        nc.tensor.matmul(cp[:, nsl], lhsT=ident_f8,
                                         rhs=rbf2[:, mt % 2, nsl],
                                         start=False, stop=(nck == 1))
                    if mt % 2 == 1:
                        del rbf_t[(s, hp, q)]
                    if mt % 2 == 0:
                        ab_t[(s, hp, q)] = abp.tile([P, 2, T], BF16,
                                                    tag="ab", name="ab")
                    half = ab_t[(s, hp, q)][:, mt % 2, :]
                    ssum = ssp.tile([P, 1], F32, tag="ssum", name="ssum")
                    nc.scalar.activation(out=half, in_=cp, func=EXPF,
                                         scale=0.125, accum_out=ssum)
                    ssum_t[(s, hp, mt)] = ssum

                def stage_n(s, hp, mt):
                    # normalize A rows by 1/rowsum; xbar at pair boundary
                    q = mt // 2
                    ssum = ssum_t.pop((s, hp, mt))
                    nc.vector.reciprocal(out=ssum, in_=ssum)
                    half = ab_t[(s, hp, q)][:, mt % 2, :]
                    nc.vector.tensor_scalar_mul(out=half, in0=half, scalar1=ssum)
                    if mt % 2 == 1:
                        at4 = at_t[(s, hp)]
                        nc.sync.dma_start_transpose(
                            out=at4[:, 2 * q:2 * q + 2, :, :],
                            in_=ab_t.pop((s, hp, q)))

                def stage_d(s):
                    # AV for both heads, col-paired: hp0 -> psum rows 0:64,
                    # hp1 -> rows 64:128 (concurrent col strips).  avps is one
                    # [P, T] tile from the shared psC pool (banks time-shared
                    # with the content tiles).
                    b, hh = sdec(s)
                    avps = psC.tile([P, T], F32, tag="cp", name="avps")
                    for nt in range(NT8):
                        for hp in range(2):
                            h = 2 * hh + hp
                            po = hp * 64
                            for mc in range(2):
                                nc.tensor.matmul(
                                    avps[po:po + 64, mc * 512:(mc + 1) * 512],
                                    lhsT=v_sb[b][:, nt, h * HS:(h + 1) * HS],
                                    rhs=at_t[(s, hp)][:, 4 * mc:4 * mc + 4, nt, :],
                                    start=(nt == 0), stop=(nt == NT8 - 1))
                    for hp in range(2):
                        del at_t[(s, hp)]
                    for mc in range(2):
                        nc.vector.tensor_copy(
                            out=avT[hh][b][:, mc * 512:(mc + 1) * 512],
                            in_=avps[:, mc * 512:(mc + 1) * 512])

                def out_proj(b):
                    with tc.tile_pool(name=f"ph4_{b}", bufs=3) as sb4, \
                         tc.tile_pool(name=f"ps4_{b}", bufs=2, space="PSUM") as ps4:
                        for mt8 in range(NT8):
                            mt = b * NT8 + mt8
                            op = ps4.tile([P, DM], F32, tag="op")
                            for sc in range(2):
                                nc.tensor.matmul(
                                    op,
                                    lhsT=avT[sc][b][:, mt8 * P:(mt8 + 1) * P],
                                    rhs=wo_sb[sc],
                                    start=(sc == 0), stop=(sc == 1))
                            ot = sb4.tile([P, DM], F32, tag="ot")
                            nc.vector.scalar_tensor_tensor(
                                out=ot, in0=op, scalar=0.0, in1=x_res[:, mt, :],
                                op0=mybir.AluOpType.bypass, op1=mybir.AluOpType.add)
                            nc.vector.tensor_tensor(out=ot, in0=ot, in1=bo_b,
                                                    op=mybir.AluOpType.add)
                            nc.scalar.dma_start(out=out[mt * P:(mt + 1) * P, :],
                                                in_=ot)

                for step in range(NST + 2):
                    if 0 <= step - 1 < NST:
                        for hp in range(2):
                            at_t[(step - 1, hp)] = atp.tile(
                                [P, NT8, NT8, P], BF16, tag="at", name="at")
                    # sweep 1: content/exp for hp0 of step-1, pos for step
                    for mt in range(NT8):
                        if 0 <= step - 1 < NST:
                            stage_c(step - 1, 0, mt)
                            if mt >= 2:
                                stage_n(step - 1, 0, mt - 2)
                        if step < NST:
                            stage_a_mm(step, mt)
                            if mt % 2 == 1:
                                stage_a_wr(step, mt // 2)
                    if 0 <= step - 1 < NST:
                        stage_n(step - 1, 0, NT8 - 2)
                        stage_n(step - 1, 0, NT8 - 1)
                    # sweep 2: content/exp for hp1 of step-1
                    for mt in range(NT8):
                        if 0 <= step - 1 < NST:
                            stage_c(step - 1, 1, mt)
                            if mt >= 2:
                                stage_n(step - 1, 1, mt - 2)
                    if 0 <= step - 1 < NST:
                        stage_n(step - 1, 1, NT8 - 2)
                        stage_n(step - 1, 1, NT8 - 1)
                    if step - 2 >= 0:
                        stage_d(step - 2)

            # ---------------- phase 4: out-proj + residual ----------------
            out_proj(0)
            out_proj(1)
    nc.finalize()
    return nc


_NC = None


def make_in_maps(inputs):
    f = lambda a: np.ascontiguousarray(np.asarray(a, dtype=np.float32))
    x = f(inputs["inputs"]).reshape(B, T, DM)
    pos = f(inputs["pos_enc"]).reshape(B, T, DM)
    wq0 = f(inputs["Wq"]).reshape(DM, DM)
    wk0 = f(inputs["Wk"]).reshape(DM, DM)
    wv0 = f(inputs["Wv"]).reshape(DM, DM)
    wp = f(inputs["Wp"]).reshape(DM, DM)
    wo = f(inputs["Wo"]).reshape(DM, DM)
    gamma = f(inputs["gamma"]).reshape(DM, 1)
    beta = f(inputs["beta"]).reshape(DM)
    # fold LN's gamma into the x-side weights, beta into the projection biases,
    # and bv through softmax (rows sum to 1) into the output bias
    wq, wk, wv = gamma * wq0, gamma * wk0, gamma * wv0
    bqu = (f(inputs["bq"]).reshape(DM) + f(inputs["pos_bias_u"]).reshape(DM)
           + beta @ wq0)
    bqv = (f(inputs["bq"]).reshape(DM) + f(inputs["pos_bias_v"]).reshape(DM)
           + beta @ wq0)
    bk = f(inputs["bk"]).reshape(DM) + beta @ wk0
    bv_eff = f(inputs["bv"]).reshape(DM) + beta @ wv0
    bo = f(inputs["bo"]) + bv_eff @ wo
    shared = dict(
        wq=wq, wk=wk, wv=wv, wp=wp, wo=wo,
        bqu=bqu, bqv=bqv, bk=bk, bo=bo,
    )
    in_maps = []
    for c in range(NCORES):
        sl = slice(c * BL, (c + 1) * BL)
        in_maps.append(dict(
            x=np.ascontiguousarray(x[sl].reshape(M, DM)),
            pos=np.ascontiguousarray(pos[sl].reshape(M, DM)),
            **shared,
        ))
    return in_maps


def kernel(**inputs) -> np.ndarray:
    global _NC
    if _NC is None:
        _NC = build_bass()
    in_maps = make_in_maps(inputs)
    res = run_bass_kernel_spmd(_NC, in_maps, core_ids=list(range(NCORES)))
    outs = [r["out"].reshape(BL, T, DM) for r in res.results]
    return np.concatenate(outs, axis=0)
